# revision 1
# baseline (speedup 1.0000x reference)
"""Causal attention with RoPE on 8 Trainium2 NeuronCores.

Tensor-parallel over heads: core c owns heads [2c, 2c+2). Each core computes
its heads' Q/K/V projections (fp32r matmuls), RoPE, causal attention in a
transposed layout (keys on partitions), and a partial output projection
through its slice of Wo. The 8 partial outputs are summed on the host.

Layout notes:
  - x is passed transposed (xT [D, B*S]) so d_model lands on partitions for
    all projection matmuls.
  - Q/K are produced transposed (QT/KT [head_dim, S]); scores are computed
    transposed (scoresT [k, q]) so the attn@V contraction has keys on
    partitions for both operands. Softmax denominators come from an M=1
    matmul with a ones vector; normalization happens on the attention output
    tiles via a gpsimd partition-broadcast of 1/denom.
  - All matmul inputs are float32r (fp32 truncated to fp22 by the PE), which
    streams at full speed (1 cycle/row) instead of fp32's 1/4 rate.
  - The causal structure lets attention for query block qj start as soon as
    projections for column block cb=qj are done, so each iteration runs
    proj(cb) -> attention(qj=cb) -> output rows of qj; engines stay mixed
    and the DMA-paced warmup overlaps compute.
"""
import numpy as np

import concourse.bacc as bacc
import concourse.bass as bass
import concourse.tile as tile
import concourse.mybir as mybir
from concourse.bass_utils import run_bass_kernel_spmd

AF = mybir.ActivationFunctionType
F32 = mybir.dt.float32
F32R = mybir.dt.float32r

P = 128            # partitions
DH = 128           # head dim
D = 2048           # d_model
S = 2048           # sequence length
B = 2              # batch
NCORES = 8
HL = 2             # heads per core
LF = HL * DH       # 256 local head features
KC = D // P        # 16 d_model chunks
NCB = S // 512     # 4 column blocks of 512 positions per batch
NKB = S // P       # 16 key blocks per batch
NNT = D // 512     # 4 output column tiles
ROWS = B * S
SCALE = float(1.0 / np.sqrt(DH))

_PROG = None


def _emit(nc, sbp, psp, t):
    xT, wqT, wkT, wvT, woT, cosT, sinT, bandT, onesT, out = (
        t["xT"], t["wqT"], t["wkT"], t["wvT"], t["woT"], t["cosT"], t["sinT"],
        t["bandT"], t["onesT"], t["out"])

    # ---------------- constants ----------------
    wq = sbp.tile([P, KC * LF], F32R, name="wq")
    wk = sbp.tile([P, KC * LF], F32R, name="wk")
    wv = sbp.tile([P, KC * LF], F32R, name="wv")
    wo = sbp.tile([P, HL * D], F32R, name="wo")
    cos = sbp.tile([DH, S], F32, name="cos")
    sin = sbp.tile([DH, S], F32, name="sin")
    band = sbp.tile([P, 640], F32, name="band")
    ones = sbp.tile([P, 1], F32R, name="ones")

    # DMA transfer bandwidth is shared, so order transfers by first use:
    # wq/wk group g feeds matmul chunk group g, x tiles interleave, and the
    # rest (cos/sin for rope, wv for the V pass, wo much later) follows.
    def emit_wqk(g):
        gk = slice(g * 4 * P, (g + 1) * 4 * P)
        nc.sync.dma_start(
            out=wq[:, g * 4 * LF:(g + 1) * 4 * LF],
            in_=wqT[gk, :].bitcast(F32R).rearrange("(kc p) f -> p kc f", p=P))
        nc.scalar.dma_start(
            out=wk[:, g * 4 * LF:(g + 1) * 4 * LF],
            in_=wkT[gk, :].bitcast(F32R).rearrange("(kc p) f -> p kc f", p=P))

    # chunk 0 of wq/wk alone first: the very first matmul needs only
    # wq[0]+wk[0]+x[0] (~0.5 MB), not the full first group
    nc.sync.dma_start(out=wq[:, 0:LF], in_=wqT[0:P, :].bitcast(F32R))
    nc.scalar.dma_start(out=wk[:, 0:LF], in_=wkT[0:P, :].bitcast(F32R))

    def emit_wqk_rest0():
        gk = slice(P, 4 * P)
        nc.sync.dma_start(
            out=wq[:, LF:4 * LF],
            in_=wqT[gk, :].bitcast(F32R).rearrange("(kc p) f -> p kc f", p=P))
        nc.scalar.dma_start(
            out=wk[:, LF:4 * LF],
            in_=wkT[gk, :].bitcast(F32R).rearrange("(kc p) f -> p kc f", p=P))

    nc.scalar.dma_start(out=cos, in_=cosT[:, :])
    nc.scalar.dma_start(out=sin, in_=sinT[:, :])

    def deferred_consts():
        nc.gpsimd.dma_start(out=band, in_=bandT[:, :])
        nc.gpsimd.dma_start(out=ones, in_=onesT[:, :].bitcast(F32R))
        for g in range(4):
            gk = slice(g * 4 * P, (g + 1) * 4 * P)
            nc.gpsimd.dma_start(
                out=wv[:, g * 4 * LF:(g + 1) * 4 * LF],
                in_=wvT[gk, :].bitcast(F32R).rearrange("(kc p) f -> p kc f",
                                                       p=P))
        # wo in nt-halves so the first Wo matmuls (nt 0/1) unblock early
        for half in range(2):
            for h in range(HL):
                nc.gpsimd.dma_start(
                    out=wo[:, h * D + half * 1024: h * D + (half + 1) * 1024],
                    in_=woT[h * P:(h + 1) * P,
                            half * 1024:(half + 1) * 1024].bitcast(F32R))

    emit_wo = _make_emit_wo(nc, sbp, psp, wo, out)
    emit_attn = _make_emit_attn(nc, sbp, psp, band, ones)

    for b in range(B):
        qt = sbp.tile([P, HL * S], F32R, name=f"qt{b}", tag="qt")
        kt = sbp.tile([P, HL * S], F32R, name=f"kt{b}", tag="kt")
        vsb = sbp.tile([P, NKB * LF], F32R, name=f"v{b}", tag="v")
        ot = sbp.tile([P, HL * S], F32R, name=f"ot{b}", tag="ot")

        for cb in range(NCB):
            # ---------------- x loads ----------------
            xtg = []
            first = b == 0 and cb == 0
            for g in range(4):
                xt = sbp.tile([P, 4 * 512], F32R, name=f"xt{b}_{cb}_{g}",
                              tag="xt", bufs=5)
                eng = nc.sync if g % 2 == 0 else nc.scalar
                src = xT[g * 4 * P:(g + 1) * 4 * P,
                         b * S + cb * 512: b * S + (cb + 1) * 512].bitcast(F32R)
                if first and g == 0:
                    # split so the kc=0 slice lands first
                    nc.sync.dma_start(
                        out=xt[:, 0:512], in_=xT[0:P, 0:512].bitcast(F32R))
                    emit_wqk_rest0()
                    nc.sync.dma_start(
                        out=xt[:, 512:2048],
                        in_=xT[P:4 * P, 0:512].bitcast(F32R)
                            .rearrange("(kc p) s -> p kc s", p=P))
                else:
                    eng.dma_start(
                        out=xt,
                        in_=src.rearrange("(kc p) s -> p kc s", p=P))
                xtg.append(xt)
                if first and g < 3:
                    emit_wqk(g + 1)
            if first:
                deferred_consts()
            xts = [xtg[kc // 4][:, (kc % 4) * 512:(kc % 4 + 1) * 512]
                   for kc in range(KC)]

            # ---------------- Q/K projections ----------------
            pqs = {}
            for key in ("q", "k"):
                for h in range(HL):
                    pqs[(key, h)] = psp.tile(
                        [P, 512], F32, name=f"p{key}{h}_{b}_{cb}",
                        tag=("po" if key == "q" else "pd"), bufs=2)
            for kc in range(KC):
                for key in ("q", "k"):
                    wsb = wq if key == "q" else wk
                    for h in range(HL):
                        nc.tensor.matmul(
                            pqs[(key, h)],
                            lhsT=wsb[:, kc * LF + h * DH: kc * LF + (h + 1) * DH],
                            rhs=xts[kc],
                            start=(kc == 0), stop=(kc == KC - 1))
            # RoPE drain: dst = pq*cos + rotate_half(pq)*sin (sin pre-signed)
            for key in ("q", "k"):
                dst = qt if key == "q" else kt
                for h in range(HL):
                    pq = pqs[(key, h)]
                    dsl = dst[:, h * S + cb * 512: h * S + (cb + 1) * 512]
                    cs = slice(cb * 512, (cb + 1) * 512)
                    ra = sbp.tile([P, 512], F32, name=f"ra{b}_{cb}_{key}{h}",
                                  tag="ex", bufs=4)
                    nc.vector.tensor_mul(ra, pq, cos[:, cs])
                    nc.vector.tensor_mul(dsl[0:64, :], pq[64:128, :],
                                         sin[0:64, cs])
                    nc.vector.tensor_mul(dsl[64:128, :], pq[0:64, :],
                                         sin[64:128, cs])
                    nc.vector.tensor_add(dsl, dsl.bitcast(F32), ra)

            # ---------------- V projection (natural layout) ----------------
            pvs = [psp.tile([P, LF], F32, name=f"pv{b}_{cb}_{r}", tag="ps",
                            bufs=4)
                   for r in range(4)]
            for kc in range(KC):
                for r in range(4):
                    nc.tensor.matmul(
                        pvs[r],
                        lhsT=xts[kc][:, r * P:(r + 1) * P],
                        rhs=wv[:, kc * LF:(kc + 1) * LF],
                        start=(kc == 0), stop=(kc == KC - 1))
            for r in range(4):
                kb = cb * 4 + r
                nc.scalar.copy(vsb[:, kb * LF:(kb + 1) * LF], pvs[r])

            # attention + partial output projection for this query block
            emit_attn(b, cb, qt, kt, vsb, ot)
            emit_wo(b, cb, ot)


def _make_emit_attn(nc, sbp, psp, band, ones):
    def emit_attn(b, qj, qt, kt, vsb, ot):
        # Both heads interleaved: each head's exp latency hides behind the
        # other head's matmuls.
        nkb = 4 * qj + 4
        po = {}
        pdn = {}
        for h in range(HL):
            po[h] = psp.tile([P, 512], F32, name=f"po{b}_{h}_{qj}",
                             tag="po", bufs=2)
            pdn[h] = psp.tile([1, 512], F32, name=f"pd{b}_{h}_{qj}",
                              tag="pd", bufs=2)
        exs = {}

        # For diagonal block r (kb = 4*qj + r), query columns [0, r*128) see
        # only masked keys in this block: skip them entirely — the scores
        # matmul, exp, attn@V and denominator all run on cols [r*128, 512).
        # The kb==0 matmuls always cover the full range (off=0 there), so
        # the accumulation start clears the whole bank.
        def _off(kb):
            return max(0, kb - 4 * qj) * P

        def emit_sc(h, kb):
            off = _off(kb)
            pss = psp.tile([P, 512], F32, name=f"pss{b}_{h}_{qj}_{kb}",
                           tag="ps", bufs=4)
            nc.tensor.matmul(
                pss[:, off:512],
                lhsT=kt[:, h * S + kb * P: h * S + (kb + 1) * P],
                rhs=qt[:, h * S + qj * 512 + off: h * S + (qj + 1) * 512],
                start=True, stop=True)
            ex = sbp.tile([P, 512], F32R, name=f"ex{b}_{h}_{qj}_{kb}",
                          tag="ex", bufs=4)
            nc.scalar.activation(ex[:, off:512], pss[:, off:512], AF.Exp,
                                 scale=SCALE)
            if kb >= 4 * qj:
                # upper-triangle mask on the diagonal 128-block
                nc.vector.tensor_mul(
                    ex[:, off:off + P], ex.bitcast(F32)[:, off:off + P],
                    band[:, 512:640])
            exs[(h, kb)] = ex

        def emit_av(h, kb, last):
            off = _off(kb)
            nc.tensor.matmul(
                po[h][:, off:512],
                lhsT=vsb[:, kb * LF + h * DH: kb * LF + h * DH + DH],
                rhs=exs[(h, kb)][:, off:512], start=(kb == 0), stop=last)
            nc.tensor.matmul(
                pdn[h][:, off:512], lhsT=ones,
                rhs=exs[(h, kb)][:, off:512], start=(kb == 0), stop=last)

        emit_sc(0, 0)
        emit_sc(1, 0)
        for kb in range(nkb):
            for h in range(HL):
                if kb + 1 < nkb:
                    emit_sc(h, kb + 1)
                emit_av(h, kb, last=(kb == nkb - 1))

        for h in range(HL):
            recip = sbp.tile([1, 512], F32, name=f"rc{b}_{h}_{qj}",
                             tag="rc", bufs=1)
            nc.vector.reciprocal(recip, pdn[h])
            bc = sbp.tile([P, 512], F32, name=f"bc{b}_{h}_{qj}",
                          tag="bc", bufs=1)
            nc.gpsimd.partition_broadcast(bc, recip)
            nc.vector.tensor_mul(
                ot[:, h * S + qj * 512: h * S + (qj + 1) * 512], po[h], bc)
    return emit_attn


def _make_emit_wo(nc, sbp, psp, wo, out):
    def emit_wo(b, qj, ot):
        for qc in range(4 * qj, 4 * qj + 4):
            st = None
            for nt in range(NNT):
                pw = psp.tile([P, 512], F32, name=f"pw{b}_{qc}_{nt}",
                              tag="ps", bufs=4)
                for h in range(HL):
                    nc.tensor.matmul(
                        pw,
                        lhsT=ot[:, h * S + qc * P: h * S + (qc + 1) * P],
                        rhs=wo[:, h * D + nt * 512: h * D + (nt + 1) * 512],
                        start=(h == 0), stop=(h == HL - 1))
                if nt % 2 == 0:
                    st = sbp.tile([P, 1024], F32, name=f"st{b}_{qc}_{nt}",
                                  tag="st", bufs=2)
                    nc.scalar.copy(st[:, 0:512], pw)
                else:
                    nc.vector.tensor_copy(st[:, 512:1024], pw)
                    nc.sync.dma_start(
                        out=out[b * S + qc * P: b * S + (qc + 1) * P,
                                (nt - 1) * 512:(nt + 1) * 512],
                        in_=st)
    return emit_wo


def _build(loop_n=0):
    nc = bacc.Bacc("TRN2", target_bir_lowering=False, debug=False)
    t = {}
    t["xT"] = nc.dram_tensor("xT", [D, ROWS], F32, kind="ExternalInput")
    t["wqT"] = nc.dram_tensor("wqT", [D, LF], F32, kind="ExternalInput")
    t["wkT"] = nc.dram_tensor("wkT", [D, LF], F32, kind="ExternalInput")
    t["wvT"] = nc.dram_tensor("wvT", [D, LF], F32, kind="ExternalInput")
    t["woT"] = nc.dram_tensor("woT", [LF, D], F32, kind="ExternalInput")
    t["cosT"] = nc.dram_tensor("cosT", [DH, S], F32, kind="ExternalInput")
    t["sinT"] = nc.dram_tensor("sinT", [DH, S], F32, kind="ExternalInput")
    t["bandT"] = nc.dram_tensor("bandT", [P, 640], F32, kind="ExternalInput")
    t["onesT"] = nc.dram_tensor("onesT", [P, 1], F32, kind="ExternalInput")
    t["out"] = nc.dram_tensor("out", [ROWS, D], F32, kind="ExternalOutput")
    with tile.TileContext(nc) as tc:
        with tc.tile_pool(name="sb", bufs=1) as sbp, \
             tc.tile_pool(name="ps", bufs=4, space="PSUM") as psp:
            if loop_n:
                with tc.For_i(0, loop_n, 1,
                              hint_engines=(mybir.EngineType.PE,
                                            mybir.EngineType.Activation,
                                            mybir.EngineType.DVE)):
                    _emit(nc, sbp, psp, t)
            else:
                _emit(nc, sbp, psp, t)
    nc.compile()
    return nc


def _tables():
    half = np.arange(0, DH, 2, dtype=np.float32) / np.float32(DH)
    inv_freq = (np.float32(1.0) / (np.float32(10000.0) ** half)).astype(np.float32)
    pos = np.arange(S, dtype=np.float32)
    freqs = np.outer(pos, inv_freq).astype(np.float32)        # [S, 64]
    emb = np.concatenate([freqs, freqs], axis=1)              # [S, DH]
    cosT = np.ascontiguousarray(np.cos(emb).T).astype(np.float32)
    sinT = np.sin(emb).T.astype(np.float32).copy()
    sinT[0:DH // 2, :] *= np.float32(-1.0)                    # pre-signed
    sinT = np.ascontiguousarray(sinT)
    # band[kl, c] = 1 iff c >= kl + 512; slice [512-r*128 : 512-r*128+(r+1)*128]
    # is the mask for diagonal block r (zeros, then upper-triangle)
    kl = np.arange(P)[:, None]
    c = np.arange(640)[None, :]
    bandT = (c >= kl + 512).astype(np.float32)
    onesT = np.ones((P, 1), np.float32)
    return cosT, sinT, bandT, onesT


def _run(inputs, trace=False, **kw):
    global _PROG
    q = np.asarray(inputs["query"], dtype=np.float32)
    Wq = np.asarray(inputs["Wq"], dtype=np.float32)
    Wk = np.asarray(inputs["Wk"], dtype=np.float32)
    Wv = np.asarray(inputs["Wv"], dtype=np.float32)
    Wo = np.asarray(inputs["Wo"], dtype=np.float32)
    if _PROG is None:
        _PROG = _build()
    xT = np.ascontiguousarray(q.reshape(ROWS, D).T)
    cosT, sinT, bandT, onesT = _tables()
    in_maps = []
    for ci in range(NCORES):
        rs = slice(ci * LF, (ci + 1) * LF)
        in_maps.append({
            "xT": xT,
            "wqT": np.ascontiguousarray(Wq[rs, :].T),
            "wkT": np.ascontiguousarray(Wk[rs, :].T),
            "wvT": np.ascontiguousarray(Wv[rs, :].T),
            "woT": np.ascontiguousarray(Wo[:, rs].T),
            "cosT": cosT, "sinT": sinT, "bandT": bandT, "onesT": onesT,
        })
    res = run_bass_kernel_spmd(_PROG, in_maps, core_ids=list(range(NCORES)),
                               trace=trace, **kw)
    acc = np.zeros((ROWS, D), np.float64)
    for r in res.results:
        acc += r["out"]
    return acc.astype(np.float32).reshape(B, S, D), res


def kernel(query, Wq, Wk, Wv, Wo):
    out, _ = _run(dict(query=query, Wq=Wq, Wk=Wk, Wv=Wv, Wo=Wo))
    return out



# revision 18
# speedup vs baseline: 1.0011x; 1.0011x over previous
"""Causal attention with RoPE on 8 Trainium2 NeuronCores.

Tensor-parallel over heads: core c owns heads [2c, 2c+2). Each core computes
its heads' Q/K/V projections (fp32r matmuls), RoPE, causal attention in a
transposed layout (keys on partitions), and a partial output projection
through its slice of Wo. The 8 partial outputs are summed on the host.

Layout notes:
  - x is passed transposed (xT [D, B*S]) so d_model lands on partitions for
    all projection matmuls.
  - Q/K are produced transposed (QT/KT [head_dim, S]); scores are computed
    transposed (scoresT [k, q]) so the attn@V contraction has keys on
    partitions for both operands. Softmax denominators come from an M=1
    matmul with a ones vector; normalization happens on the attention output
    tiles via a gpsimd partition-broadcast of 1/denom.
  - All matmul inputs are float32r (fp32 truncated to fp22 by the PE), which
    streams at full speed (1 cycle/row) instead of fp32's 1/4 rate.
  - The causal structure lets attention for query block qj start as soon as
    projections for column block cb=qj are done, so each iteration runs
    proj(cb) -> attention(qj=cb) -> output rows of qj; engines stay mixed
    and the DMA-paced warmup overlaps compute.
"""
import numpy as np

import concourse.bacc as bacc
import concourse.bass as bass
import concourse.bass_isa as bass_isa
import concourse.tile as tile
import concourse.mybir as mybir
from concourse.bass_utils import run_bass_kernel_spmd

AF = mybir.ActivationFunctionType
F32 = mybir.dt.float32
F32R = mybir.dt.float32r
BF16 = mybir.dt.bfloat16

P = 128            # partitions
DH = 128           # head dim
D = 2048           # d_model
S = 2048           # sequence length
B = 2              # batch
NCORES = 8
HL = 2             # heads per core
LF = HL * DH       # 256 local head features
KC = D // P        # 16 d_model chunks
NCB = S // 512     # 4 column blocks of 512 positions per batch
NKB = S // P       # 16 key blocks per batch
NNT = D // 512     # 4 output column tiles
ROWS = B * S
SCALE = float(1.0 / np.sqrt(DH))

_PROG = None


def _emit(nc, sbp, psp, t):
    xT, wqT, wkT, wvT, woT, cosT, sinT, bandT, out = (
        t["xT"], t["wqT"], t["wkT"], t["wvT"], t["woT"], t["cosT"], t["sinT"],
        t["bandT"], t["out"])

    # ---------------- constants ----------------
    wq = sbp.tile([P, KC * LF], F32R, name="wq")
    wk = sbp.tile([P, KC * LF], F32R, name="wk")
    wv = sbp.tile([P, KC * LF], F32R, name="wv")
    wo = sbp.tile([P, HL * D], F32R, name="wo")
    cos = sbp.tile([DH, S], F32, name="cos")
    sin = sbp.tile([DH, S], F32, name="sin")
    band = sbp.tile([P, 128], F32, name="band")

    # DMA transfer bandwidth is shared, so order transfers by first use:
    # wq/wk group g feeds matmul chunk group g, x tiles interleave, and the
    # rest (cos/sin for rope, wv for the V pass, wo much later) follows.
    def emit_wqk(g):
        gk = slice(g * 4 * P, (g + 1) * 4 * P)
        nc.sync.dma_start(
            out=wq[:, g * 4 * LF:(g + 1) * 4 * LF],
            in_=wqT[gk, :].bitcast(F32R).rearrange("(kc p) f -> p kc f", p=P))
        nc.scalar.dma_start(
            out=wk[:, g * 4 * LF:(g + 1) * 4 * LF],
            in_=wkT[gk, :].bitcast(F32R).rearrange("(kc p) f -> p kc f", p=P))

    # chunk 0 of wq/wk alone first: the very first matmul needs only
    # wq[0]+wk[0]+x[0] (~0.5 MB), not the full first group
    nc.sync.dma_start(out=wq[:, 0:LF], in_=wqT[0:P, :].bitcast(F32R))
    nc.scalar.dma_start(out=wk[:, 0:LF], in_=wkT[0:P, :].bitcast(F32R))

    def emit_wqk_rest0():
        gk = slice(P, 4 * P)
        nc.sync.dma_start(
            out=wq[:, LF:4 * LF],
            in_=wqT[gk, :].bitcast(F32R).rearrange("(kc p) f -> p kc f", p=P))
        nc.scalar.dma_start(
            out=wk[:, LF:4 * LF],
            in_=wkT[gk, :].bitcast(F32R).rearrange("(kc p) f -> p kc f", p=P))

    nc.scalar.dma_start(out=cos, in_=cosT[:, :])
    nc.scalar.dma_start(out=sin, in_=sinT[:, :])

    def deferred_consts():
        nc.gpsimd.dma_start(out=band, in_=bandT[:, :])
        for g in range(4):
            gk = slice(g * 4 * P, (g + 1) * 4 * P)
            nc.gpsimd.dma_start(
                out=wv[:, g * 4 * LF:(g + 1) * 4 * LF],
                in_=wvT[gk, :].bitcast(F32R).rearrange("(kc p) f -> p kc f",
                                                       p=P))
        # wo in nt-halves so the first Wo matmuls (nt 0/1) unblock early
        for half in range(2):
            for h in range(HL):
                nc.gpsimd.dma_start(
                    out=wo[:, h * D + half * 1024: h * D + (half + 1) * 1024],
                    in_=woT[h * P:(h + 1) * P,
                            half * 1024:(half + 1) * 1024].bitcast(F32R))

    emit_wo = _make_emit_wo(nc, sbp, psp, wo, out)
    emit_attn = _make_emit_attn(nc, sbp, psp, band)

    for b in range(B):
        qt = sbp.tile([P, HL * S], F32R, name=f"qt{b}", tag="qt")
        kt = sbp.tile([P, HL * S], F32R, name=f"kt{b}", tag="kt")
        vsb = sbp.tile([P, NKB * LF], F32R, name=f"v{b}", tag="v")
        ot = sbp.tile([P, HL * S], F32R, name=f"ot{b}", tag="ot")

        for cb in range(NCB):
            # ---------------- x loads ----------------
            xtg = []
            first = b == 0 and cb == 0
            for g in range(4):
                xt = sbp.tile([P, 4 * 512], F32R, name=f"xt{b}_{cb}_{g}",
                              tag="xt", bufs=5)
                eng = nc.sync if g % 2 == 0 else nc.scalar
                src = xT[g * 4 * P:(g + 1) * 4 * P,
                         b * S + cb * 512: b * S + (cb + 1) * 512].bitcast(F32R)
                if first and g == 0:
                    # split so the kc=0 slice lands first
                    nc.sync.dma_start(
                        out=xt[:, 0:512], in_=xT[0:P, 0:512].bitcast(F32R))
                    emit_wqk_rest0()
                    nc.sync.dma_start(
                        out=xt[:, 512:2048],
                        in_=xT[P:4 * P, 0:512].bitcast(F32R)
                            .rearrange("(kc p) s -> p kc s", p=P))
                else:
                    eng.dma_start(
                        out=xt,
                        in_=src.rearrange("(kc p) s -> p kc s", p=P))
                xtg.append(xt)
                if first and g < 3:
                    emit_wqk(g + 1)
            if first:
                deferred_consts()
            xts = [xtg[kc // 4][:, (kc % 4) * 512:(kc % 4 + 1) * 512]
                   for kc in range(KC)]

            # ---------------- Q/K projections ----------------
            pqs = {}
            for key in ("q", "k"):
                for h in range(HL):
                    pqs[(key, h)] = psp.tile(
                        [P, 512], F32, name=f"p{key}{h}_{b}_{cb}",
                        tag=("po" if key == "q" else "pd"), bufs=2)
            for kc in range(KC):
                for key in ("q", "k"):
                    wsb = wq if key == "q" else wk
                    for h in range(HL):
                        nc.tensor.matmul(
                            pqs[(key, h)],
                            lhsT=wsb[:, kc * LF + h * DH: kc * LF + (h + 1) * DH],
                            rhs=xts[kc],
                            start=(kc == 0), stop=(kc == KC - 1))
            # RoPE drain: dst = pq*cos + rotate_half(pq)*sin (sin pre-signed)
            for key in ("q", "k"):
                dst = qt if key == "q" else kt
                for h in range(HL):
                    pq = pqs[(key, h)]
                    dsl = dst[:, h * S + cb * 512: h * S + (cb + 1) * 512]
                    cs = slice(cb * 512, (cb + 1) * 512)
                    ra = sbp.tile([P, 512], F32, name=f"ra{b}_{cb}_{key}{h}",
                                  tag="ex", bufs=4)
                    nc.vector.tensor_mul(ra, pq, cos[:, cs])
                    nc.vector.tensor_mul(dsl[0:64, :], pq[64:128, :],
                                         sin[0:64, cs])
                    nc.vector.tensor_mul(dsl[64:128, :], pq[0:64, :],
                                         sin[64:128, cs])
                    nc.vector.tensor_add(dsl, dsl.bitcast(F32), ra)

            # ---------------- V projection (natural layout) ----------------
            pvs = [psp.tile([P, LF], F32, name=f"pv{b}_{cb}_{r}", tag="ps",
                            bufs=4)
                   for r in range(4)]
            for kc in range(KC):
                for r in range(4):
                    nc.tensor.matmul(
                        pvs[r],
                        lhsT=xts[kc][:, r * P:(r + 1) * P],
                        rhs=wv[:, kc * LF:(kc + 1) * LF],
                        start=(kc == 0), stop=(kc == KC - 1))
            for r in range(4):
                kb = cb * 4 + r
                nc.scalar.copy(vsb[:, kb * LF:(kb + 1) * LF], pvs[r])

            # attention + partial output projection for this query block
            emit_attn(b, cb, qt, kt, vsb, ot)
            emit_wo(b, cb, ot)


def _make_emit_attn(nc, sbp, psp, band):
    def emit_attn(b, qj, qt, kt, vsb, ot):
        # Both heads interleaved: each head's exp latency hides behind the
        # other head's matmuls.
        nkb = 4 * qj + 4
        po = {}
        acc = {}
        for h in range(HL):
            po[h] = psp.tile([P, 512], F32, name=f"po{b}_{h}_{qj}",
                             tag="po", bufs=2)
            # running key-sum of exp (DVE); partition-reduced once at the end
            acc[h] = sbp.tile([P, 512], F32, name=f"ac{b}_{h}_{qj}",
                              tag="ac", bufs=2)
        exs = {}

        # For diagonal block r (kb = 4*qj + r), query columns [0, r*128) see
        # only masked keys in this block: skip them entirely — the scores
        # matmul, exp, attn@V and denominator all run on cols [r*128, 512).
        # The kb==0 matmuls always cover the full range (off=0 there), so
        # the accumulation start clears the whole bank.
        def _off(kb):
            return max(0, kb - 4 * qj) * P

        def emit_sc(h, kb):
            off = _off(kb)
            pss = psp.tile([P, 512], F32, name=f"pss{b}_{h}_{qj}_{kb}",
                           tag="ps", bufs=4)
            nc.tensor.matmul(
                pss[:, off:512],
                lhsT=kt[:, h * S + kb * P: h * S + (kb + 1) * P],
                rhs=qt[:, h * S + qj * 512 + off: h * S + (qj + 1) * 512],
                start=True, stop=True)
            ex = sbp.tile([P, 512], F32R, name=f"ex{b}_{h}_{qj}_{kb}",
                          tag="ex", bufs=4)
            nc.scalar.activation(ex[:, off:512], pss[:, off:512], AF.Exp,
                                 scale=SCALE)
            if kb >= 4 * qj:
                # upper-triangle mask on the diagonal 128-block
                nc.vector.tensor_mul(
                    ex[:, off:off + P], ex.bitcast(F32)[:, off:off + P],
                    band[:, 0:128])
            exs[(h, kb)] = ex

        def emit_av(h, kb, last):
            off = _off(kb)
            nc.tensor.matmul(
                po[h][:, off:512],
                lhsT=vsb[:, kb * LF + h * DH: kb * LF + h * DH + DH],
                rhs=exs[(h, kb)][:, off:512], start=(kb == 0), stop=last)
            # denominator accumulate on DVE (columns < off see only masked
            # keys in this block, so skipping them keeps acc correct)
            if kb == 0:
                nc.vector.tensor_copy(acc[h], exs[(h, kb)].bitcast(F32))
            else:
                nc.vector.tensor_add(
                    acc[h][:, off:512], acc[h][:, off:512],
                    exs[(h, kb)].bitcast(F32)[:, off:512])

        emit_sc(0, 0)
        emit_sc(1, 0)
        for kb in range(nkb):
            for h in range(HL):
                if kb + 1 < nkb:
                    emit_sc(h, kb + 1)
                emit_av(h, kb, last=(kb == nkb - 1))

        for h in range(HL):
            ar = sbp.tile([P, 512], F32, name=f"ar{b}_{h}_{qj}",
                          tag="ar", bufs=2)
            nc.gpsimd.partition_all_reduce(ar, acc[h], channels=P,
                                           reduce_op=bass_isa.ReduceOp.add)
            nc.vector.reciprocal(ar, ar)
            nc.vector.tensor_mul(
                ot[:, h * S + qj * 512: h * S + (qj + 1) * 512], po[h], ar)
    return emit_attn


def _make_emit_wo(nc, sbp, psp, wo, out):
    def emit_wo(b, qj, ot):
        for qc in range(4 * qj, 4 * qj + 4):
            st = None
            for nt in range(NNT):
                pw = psp.tile([P, 512], F32, name=f"pw{b}_{qc}_{nt}",
                              tag="ps", bufs=4)
                for h in range(HL):
                    nc.tensor.matmul(
                        pw,
                        lhsT=ot[:, h * S + qc * P: h * S + (qc + 1) * P],
                        rhs=wo[:, h * D + nt * 512: h * D + (nt + 1) * 512],
                        start=(h == 0), stop=(h == HL - 1))
                if nt % 2 == 0:
                    st = sbp.tile([P, 1024], BF16, name=f"st{b}_{qc}_{nt}",
                                  tag="st", bufs=2)
                    nc.scalar.copy(st[:, 0:512], pw)
                else:
                    nc.vector.tensor_copy(st[:, 512:1024], pw)
                    nc.sync.dma_start(
                        out=out[b * S + qc * P: b * S + (qc + 1) * P,
                                (nt - 1) * 512:(nt + 1) * 512],
                        in_=st)
    return emit_wo


def _build(loop_n=0):
    nc = bacc.Bacc("TRN2", target_bir_lowering=False, debug=False)
    t = {}
    t["xT"] = nc.dram_tensor("xT", [D, ROWS], F32, kind="ExternalInput")
    t["wqT"] = nc.dram_tensor("wqT", [D, LF], F32, kind="ExternalInput")
    t["wkT"] = nc.dram_tensor("wkT", [D, LF], F32, kind="ExternalInput")
    t["wvT"] = nc.dram_tensor("wvT", [D, LF], F32, kind="ExternalInput")
    t["woT"] = nc.dram_tensor("woT", [LF, D], F32, kind="ExternalInput")
    t["cosT"] = nc.dram_tensor("cosT", [DH, S], F32, kind="ExternalInput")
    t["sinT"] = nc.dram_tensor("sinT", [DH, S], F32, kind="ExternalInput")
    t["bandT"] = nc.dram_tensor("bandT", [P, 128], F32, kind="ExternalInput")
    t["out"] = nc.dram_tensor("out", [ROWS, D], BF16, kind="ExternalOutput")
    with tile.TileContext(nc) as tc:
        with tc.tile_pool(name="sb", bufs=1) as sbp, \
             tc.tile_pool(name="ps", bufs=4, space="PSUM") as psp:
            if loop_n:
                with tc.For_i(0, loop_n, 1,
                              hint_engines=(mybir.EngineType.PE,
                                            mybir.EngineType.Activation,
                                            mybir.EngineType.DVE)):
                    _emit(nc, sbp, psp, t)
            else:
                _emit(nc, sbp, psp, t)
    nc.compile()
    return nc


def _tables():
    half = np.arange(0, DH, 2, dtype=np.float32) / np.float32(DH)
    inv_freq = (np.float32(1.0) / (np.float32(10000.0) ** half)).astype(np.float32)
    pos = np.arange(S, dtype=np.float32)
    freqs = np.outer(pos, inv_freq).astype(np.float32)        # [S, 64]
    emb = np.concatenate([freqs, freqs], axis=1)              # [S, DH]
    cosT = np.ascontiguousarray(np.cos(emb).T).astype(np.float32)
    sinT = np.sin(emb).T.astype(np.float32).copy()
    sinT[0:DH // 2, :] *= np.float32(-1.0)                    # pre-signed
    sinT = np.ascontiguousarray(sinT)
    # band[kl, c] = 1 iff c >= kl: the upper-triangle mask every diagonal
    # 128-block uses (query col off+c vs key row off+kl differ by c-kl)
    kl = np.arange(P)[:, None]
    c = np.arange(P)[None, :]
    bandT = (c >= kl).astype(np.float32)
    onesT = np.ones((P, 1), np.float32)
    return cosT, sinT, bandT, onesT


def _run(inputs, trace=False, **kw):
    global _PROG
    q = np.asarray(inputs["query"], dtype=np.float32)
    Wq = np.asarray(inputs["Wq"], dtype=np.float32)
    Wk = np.asarray(inputs["Wk"], dtype=np.float32)
    Wv = np.asarray(inputs["Wv"], dtype=np.float32)
    Wo = np.asarray(inputs["Wo"], dtype=np.float32)
    if _PROG is None:
        _PROG = _build()
    xT = np.ascontiguousarray(q.reshape(ROWS, D).T)
    cosT, sinT, bandT, onesT = _tables()
    in_maps = []
    for ci in range(NCORES):
        rs = slice(ci * LF, (ci + 1) * LF)
        in_maps.append({
            "xT": xT,
            "wqT": np.ascontiguousarray(Wq[rs, :].T),
            "wkT": np.ascontiguousarray(Wk[rs, :].T),
            "wvT": np.ascontiguousarray(Wv[rs, :].T),
            "woT": np.ascontiguousarray(Wo[:, rs].T),
            "cosT": cosT, "sinT": sinT, "bandT": bandT, "onesT": onesT,
        })
    res = run_bass_kernel_spmd(_PROG, in_maps, core_ids=list(range(NCORES)),
                               trace=trace, **kw)
    acc = np.zeros((ROWS, D), np.float64)
    for r in res.results:
        acc += np.asarray(r["out"], np.float32)
    return acc.astype(np.float32).reshape(B, S, D), res


def kernel(query, Wq, Wk, Wv, Wo):
    out, _ = _run(dict(query=query, Wq=Wq, Wk=Wk, Wv=Wv, Wo=Wo))
    return out



# revision 32
# speedup vs baseline: 1.0029x; 1.0017x over previous
"""Causal attention with RoPE on 8 Trainium2 NeuronCores.

Tensor-parallel over heads: core c owns heads [2c, 2c+2). Each core computes
its heads' Q/K/V projections (fp32r matmuls), RoPE, causal attention in a
transposed layout (keys on partitions), and a partial output projection
through its slice of Wo. The 8 partial outputs are summed on the host.

Layout notes:
  - x is passed transposed (xT [D, B*S]) so d_model lands on partitions for
    all projection matmuls.
  - Q/K are produced transposed (QT/KT [head_dim, S]); scores are computed
    transposed (scoresT [k, q]) so the attn@V contraction has keys on
    partitions for both operands. Softmax denominators come from an M=1
    matmul with a ones vector; normalization happens on the attention output
    tiles via a gpsimd partition-broadcast of 1/denom.
  - All matmul inputs are float32r (fp32 truncated to fp22 by the PE), which
    streams at full speed (1 cycle/row) instead of fp32's 1/4 rate.
  - The causal structure lets attention for query block qj start as soon as
    projections for column block cb=qj are done, so each iteration runs
    proj(cb) -> attention(qj=cb) -> output rows of qj; engines stay mixed
    and the DMA-paced warmup overlaps compute.
"""
import numpy as np

import concourse.bacc as bacc
import concourse.bass as bass
import concourse.bass_isa as bass_isa
import concourse.tile as tile
import concourse.mybir as mybir
from concourse.bass_utils import run_bass_kernel_spmd

AF = mybir.ActivationFunctionType
F32 = mybir.dt.float32
F32R = mybir.dt.float32r
BF16 = mybir.dt.bfloat16

P = 128            # partitions
DH = 128           # head dim
D = 2048           # d_model
S = 2048           # sequence length
B = 2              # batch
NCORES = 8
HL = 2             # heads per core
LF = HL * DH       # 256 local head features
KC = D // P        # 16 d_model chunks
NCB = S // 512     # 4 column blocks of 512 positions per batch
NKB = S // P       # 16 key blocks per batch
NNT = D // 512     # 4 output column tiles
ROWS = B * S
SCALE = float(1.0 / np.sqrt(DH))

_PROG = None
# timing-only probes (wrong numerics): 1 = halve x DMA, 2 = skip odd-kb exp,
# 3 = skip attention entirely
PROBE = 0


def _emit(nc, sbp, psp, t):
    xT, wqT, wkT, wvT, woT, cosT, sinT, bandT, out = (
        t["xT"], t["wqT"], t["wkT"], t["wvT"], t["woT"], t["cosT"], t["sinT"],
        t["bandT"], t["out"])

    # ---------------- constants ----------------
    wq = sbp.tile([P, KC * LF], F32R, name="wq")
    wk = sbp.tile([P, KC * LF], F32R, name="wk")
    wv = sbp.tile([P, KC * LF], F32R, name="wv")
    wo = sbp.tile([P, HL * D], F32R, name="wo")
    cos = sbp.tile([DH, S], F32, name="cos")
    sin = sbp.tile([DH, S], F32, name="sin")
    band = sbp.tile([P, 128], BF16, name="band")

    # DMA transfer bandwidth is shared, so order transfers by first use:
    # wq/wk group g feeds matmul chunk group g, x tiles interleave, and the
    # rest (cos/sin for rope, wv for the V pass, wo much later) follows.
    def emit_wqk(g):
        gk = slice(g * 4 * P, (g + 1) * 4 * P)
        nc.sync.dma_start(
            out=wq[:, g * 4 * LF:(g + 1) * 4 * LF],
            in_=wqT[gk, :].bitcast(F32R).rearrange("(kc p) f -> p kc f", p=P))
        nc.scalar.dma_start(
            out=wk[:, g * 4 * LF:(g + 1) * 4 * LF],
            in_=wkT[gk, :].bitcast(F32R).rearrange("(kc p) f -> p kc f", p=P))

    # chunk 0 of wq/wk alone first: the very first matmul needs only
    # wq[0]+wk[0]+x[0] (~0.5 MB), not the full first group
    nc.sync.dma_start(out=wq[:, 0:LF], in_=wqT[0:P, :].bitcast(F32R))
    nc.scalar.dma_start(out=wk[:, 0:LF], in_=wkT[0:P, :].bitcast(F32R))

    def emit_wqk_rest0():
        gk = slice(P, 4 * P)
        nc.sync.dma_start(
            out=wq[:, LF:4 * LF],
            in_=wqT[gk, :].bitcast(F32R).rearrange("(kc p) f -> p kc f", p=P))
        nc.scalar.dma_start(
            out=wk[:, LF:4 * LF],
            in_=wkT[gk, :].bitcast(F32R).rearrange("(kc p) f -> p kc f", p=P))

    nc.scalar.dma_start(out=cos, in_=cosT[:, :])
    nc.scalar.dma_start(out=sin, in_=sinT[:, :])

    def deferred_consts():
        nc.gpsimd.dma_start(out=band, in_=bandT[:, :])
        for g in range(4):
            gk = slice(g * 4 * P, (g + 1) * 4 * P)
            nc.gpsimd.dma_start(
                out=wv[:, g * 4 * LF:(g + 1) * 4 * LF],
                in_=wvT[gk, :].bitcast(F32R).rearrange("(kc p) f -> p kc f",
                                                       p=P))
        # wo in nt-halves so the first Wo matmuls (nt 0/1) unblock early
        for half in range(2):
            for h in range(HL):
                nc.gpsimd.dma_start(
                    out=wo[:, h * D + half * 1024: h * D + (half + 1) * 1024],
                    in_=woT[h * P:(h + 1) * P,
                            half * 1024:(half + 1) * 1024].bitcast(F32R))

    emit_wo = _make_emit_wo(nc, sbp, psp, wo, out)
    emit_attn = _make_emit_attn(nc, sbp, psp, band)

    for b in range(B):
        qt = sbp.tile([P, HL * S], F32R, name=f"qt{b}", tag="qt")
        kt = sbp.tile([P, HL * S], F32R, name=f"kt{b}", tag="kt")
        vsb = sbp.tile([P, NKB * LF], BF16, name=f"v{b}", tag="v")
        ot = sbp.tile([P, HL * S], F32R, name=f"ot{b}", tag="ot")

        for cb in range(NCB):
            # ---------------- x loads ----------------
            xtg = []
            first = b == 0 and cb == 0
            ngrp = 2 if PROBE == 1 else 4
            for g in range(ngrp):
                xt = sbp.tile([P, 4 * 512], F32R, name=f"xt{b}_{cb}_{g}",
                              tag="xt", bufs=5)
                eng = nc.sync if g % 2 == 0 else nc.scalar
                src = xT[g * 4 * P:(g + 1) * 4 * P,
                         b * S + cb * 512: b * S + (cb + 1) * 512].bitcast(F32R)
                if first and g == 0:
                    # split so the kc=0 slice lands first
                    nc.sync.dma_start(
                        out=xt[:, 0:512], in_=xT[0:P, 0:512].bitcast(F32R))
                    emit_wqk_rest0()
                    nc.sync.dma_start(
                        out=xt[:, 512:2048],
                        in_=xT[P:4 * P, 0:512].bitcast(F32R)
                            .rearrange("(kc p) s -> p kc s", p=P))
                else:
                    eng.dma_start(
                        out=xt,
                        in_=src.rearrange("(kc p) s -> p kc s", p=P))
                xtg.append(xt)
                if first and g < 3:
                    emit_wqk(g + 1)
            if first:
                deferred_consts()
            xts = [xtg[(kc // 4) % ngrp][:, (kc % 4) * 512:(kc % 4 + 1) * 512]
                   for kc in range(KC)]

            # ---------------- Q/K projections ----------------
            pqs = {}
            for key in ("q", "k"):
                for h in range(HL):
                    pqs[(key, h)] = psp.tile(
                        [P, 512], F32, name=f"p{key}{h}_{b}_{cb}",
                        tag=("po" if key == "q" else "pd"), bufs=2)
            for kc in range(KC):
                for key in ("q", "k"):
                    wsb = wq if key == "q" else wk
                    for h in range(HL):
                        nc.tensor.matmul(
                            pqs[(key, h)],
                            lhsT=wsb[:, kc * LF + h * DH: kc * LF + (h + 1) * DH],
                            rhs=xts[kc],
                            start=(kc == 0), stop=(kc == KC - 1))
            # RoPE drain: dst = pq*cos + rotate_half(pq)*sin (sin pre-signed)
            for key in ("q", "k"):
                dst = qt if key == "q" else kt
                for h in range(HL):
                    pq = pqs[(key, h)]
                    dsl = dst[:, h * S + cb * 512: h * S + (cb + 1) * 512]
                    cs = slice(cb * 512, (cb + 1) * 512)
                    ra = sbp.tile([P, 512], F32, name=f"ra{b}_{cb}_{key}{h}",
                                  tag="ra", bufs=4)
                    nc.vector.tensor_mul(ra, pq, cos[:, cs])
                    nc.vector.tensor_mul(dsl[0:64, :], pq[64:128, :],
                                         sin[0:64, cs])
                    nc.vector.tensor_mul(dsl[64:128, :], pq[0:64, :],
                                         sin[64:128, cs])
                    nc.vector.tensor_add(dsl, dsl.bitcast(F32), ra)

            # ---------------- V projection (natural layout) ----------------
            pvs = [psp.tile([P, LF], F32, name=f"pv{b}_{cb}_{r}", tag="ps",
                            bufs=4)
                   for r in range(4)]
            for kc in range(KC):
                for r in range(4):
                    nc.tensor.matmul(
                        pvs[r],
                        lhsT=xts[kc][:, r * P:(r + 1) * P],
                        rhs=wv[:, kc * LF:(kc + 1) * LF],
                        start=(kc == 0), stop=(kc == KC - 1))
            for r in range(4):
                kb = cb * 4 + r
                nc.scalar.copy(vsb[:, kb * LF:(kb + 1) * LF], pvs[r])
            if PROBE == 3:
                continue

            # attention + partial output projection for this query block
            if PROBE != 3:
                emit_attn(b, cb, qt, kt, vsb, ot)
                emit_wo(b, cb, ot)


def _make_emit_attn(nc, sbp, psp, band):
    def emit_attn(b, qj, qt, kt, vsb, ot):
        # Both heads interleaved: each head's exp latency hides behind the
        # other head's matmuls.
        nkb = 4 * qj + 4
        po = {}
        acc = {}
        for h in range(HL):
            po[h] = psp.tile([P, 512], F32, name=f"po{b}_{h}_{qj}",
                             tag="po", bufs=2)
            # running key-sum of exp (DVE); partition-reduced once at the end
            acc[h] = sbp.tile([P, 512], F32, name=f"ac{b}_{h}_{qj}",
                              tag="ac", bufs=2)
        exs = {}

        # For diagonal block r (kb = 4*qj + r), query columns [0, r*128) see
        # only masked keys in this block: skip them entirely — the scores
        # matmul, exp, attn@V and denominator all run on cols [r*128, 512).
        # The kb==0 matmuls always cover the full range (off=0 there), so
        # the accumulation start clears the whole bank.
        def _off(kb):
            return max(0, kb - 4 * qj) * P

        def emit_sc(h, kb):
            off = _off(kb)
            pss = psp.tile([P, 512], F32, name=f"pss{b}_{h}_{qj}_{kb}",
                           tag="ps", bufs=4)
            nc.tensor.matmul(
                pss[:, off:512],
                lhsT=kt[:, h * S + kb * P: h * S + (kb + 1) * P],
                rhs=qt[:, h * S + qj * 512 + off: h * S + (qj + 1) * 512],
                start=True, stop=True)
            ex = sbp.tile([P, 512], BF16, name=f"ex{b}_{h}_{qj}_{kb}",
                          tag="ex", bufs=6)
            nc.scalar.activation(ex[:, off:512], pss[:, off:512], AF.Exp,
                                 scale=SCALE)
            if kb >= 4 * qj:
                # upper-triangle mask on the diagonal 128-block
                nc.vector.tensor_mul(
                    ex[:, off:off + P], ex[:, off:off + P],
                    band[:, 0:128])
            exs[(h, kb)] = ex

        def emit_av(h, kb, last):
            off = _off(kb)
            nc.tensor.matmul(
                po[h][:, off:512],
                lhsT=vsb[:, kb * LF + h * DH: kb * LF + h * DH + DH],
                rhs=exs[(h, kb)][:, off:512], start=(kb == 0), stop=last)
            # denominator accumulate on DVE (columns < off see only masked
            # keys in this block, so skipping them keeps acc correct)
            if kb == 0:
                nc.vector.tensor_copy(acc[h], exs[(h, kb)])
            else:
                nc.vector.tensor_add(
                    acc[h][:, off:512], acc[h][:, off:512],
                    exs[(h, kb)][:, off:512])

        emit_sc(0, 0)
        emit_sc(1, 0)
        for kb in range(nkb):
            for h in range(HL):
                if kb + 1 < nkb:
                    emit_sc(h, kb + 1)
                emit_av(h, kb, last=(kb == nkb - 1))

        for h in range(HL):
            ar = sbp.tile([P, 512], F32, name=f"ar{b}_{h}_{qj}",
                          tag="ar", bufs=2)
            nc.gpsimd.partition_all_reduce(ar, acc[h], channels=P,
                                           reduce_op=bass_isa.ReduceOp.add)
            nc.vector.reciprocal(ar, ar)
            nc.vector.tensor_mul(
                ot[:, h * S + qj * 512: h * S + (qj + 1) * 512], po[h], ar)
    return emit_attn


def _make_emit_wo(nc, sbp, psp, wo, out):
    def emit_wo(b, qj, ot):
        for qc in range(4 * qj, 4 * qj + 4):
            st = None
            for nt in range(NNT):
                pw = psp.tile([P, 512], F32, name=f"pw{b}_{qc}_{nt}",
                              tag="ps", bufs=4)
                for h in range(HL):
                    nc.tensor.matmul(
                        pw,
                        lhsT=ot[:, h * S + qc * P: h * S + (qc + 1) * P],
                        rhs=wo[:, h * D + nt * 512: h * D + (nt + 1) * 512],
                        start=(h == 0), stop=(h == HL - 1))
                if nt % 2 == 0:
                    st = sbp.tile([P, 1024], BF16, name=f"st{b}_{qc}_{nt}",
                                  tag="st", bufs=2)
                    nc.scalar.copy(st[:, 0:512], pw)
                else:
                    nc.vector.tensor_copy(st[:, 512:1024], pw)
                    nc.sync.dma_start(
                        out=out[b * S + qc * P: b * S + (qc + 1) * P,
                                (nt - 1) * 512:(nt + 1) * 512],
                        in_=st)
    return emit_wo


def _build(loop_n=0):
    nc = bacc.Bacc("TRN2", target_bir_lowering=False, debug=False)
    t = {}
    t["xT"] = nc.dram_tensor("xT", [D, ROWS], F32, kind="ExternalInput")
    t["wqT"] = nc.dram_tensor("wqT", [D, LF], F32, kind="ExternalInput")
    t["wkT"] = nc.dram_tensor("wkT", [D, LF], F32, kind="ExternalInput")
    t["wvT"] = nc.dram_tensor("wvT", [D, LF], F32, kind="ExternalInput")
    t["woT"] = nc.dram_tensor("woT", [LF, D], F32, kind="ExternalInput")
    t["cosT"] = nc.dram_tensor("cosT", [DH, S], F32, kind="ExternalInput")
    t["sinT"] = nc.dram_tensor("sinT", [DH, S], F32, kind="ExternalInput")
    t["bandT"] = nc.dram_tensor("bandT", [P, 128], BF16, kind="ExternalInput")
    t["out"] = nc.dram_tensor("out", [ROWS, D], BF16, kind="ExternalOutput")
    with tile.TileContext(nc) as tc:
        with tc.tile_pool(name="sb", bufs=1) as sbp, \
             tc.tile_pool(name="ps", bufs=4, space="PSUM") as psp:
            if loop_n:
                with tc.For_i(0, loop_n, 1,
                              hint_engines=(mybir.EngineType.PE,
                                            mybir.EngineType.Activation,
                                            mybir.EngineType.DVE)):
                    _emit(nc, sbp, psp, t)
            else:
                _emit(nc, sbp, psp, t)
    nc.compile()
    return nc


def _tables():
    half = np.arange(0, DH, 2, dtype=np.float32) / np.float32(DH)
    inv_freq = (np.float32(1.0) / (np.float32(10000.0) ** half)).astype(np.float32)
    pos = np.arange(S, dtype=np.float32)
    freqs = np.outer(pos, inv_freq).astype(np.float32)        # [S, 64]
    emb = np.concatenate([freqs, freqs], axis=1)              # [S, DH]
    cosT = np.ascontiguousarray(np.cos(emb).T).astype(np.float32)
    sinT = np.sin(emb).T.astype(np.float32).copy()
    sinT[0:DH // 2, :] *= np.float32(-1.0)                    # pre-signed
    sinT = np.ascontiguousarray(sinT)
    # band[kl, c] = 1 iff c >= kl: the upper-triangle mask every diagonal
    # 128-block uses (query col off+c vs key row off+kl differ by c-kl)
    kl = np.arange(P)[:, None]
    c = np.arange(P)[None, :]
    import ml_dtypes
    bandT = (c >= kl).astype(ml_dtypes.bfloat16)
    onesT = np.ones((P, 1), np.float32)
    return cosT, sinT, bandT, onesT


def _run(inputs, trace=False, **kw):
    global _PROG
    q = np.asarray(inputs["query"], dtype=np.float32)
    Wq = np.asarray(inputs["Wq"], dtype=np.float32)
    Wk = np.asarray(inputs["Wk"], dtype=np.float32)
    Wv = np.asarray(inputs["Wv"], dtype=np.float32)
    Wo = np.asarray(inputs["Wo"], dtype=np.float32)
    if _PROG is None:
        _PROG = _build()
    xT = np.ascontiguousarray(q.reshape(ROWS, D).T)
    cosT, sinT, bandT, onesT = _tables()
    in_maps = []
    for ci in range(NCORES):
        rs = slice(ci * LF, (ci + 1) * LF)
        in_maps.append({
            "xT": xT,
            "wqT": np.ascontiguousarray(Wq[rs, :].T),
            "wkT": np.ascontiguousarray(Wk[rs, :].T),
            "wvT": np.ascontiguousarray(Wv[rs, :].T),
            "woT": np.ascontiguousarray(Wo[:, rs].T),
            "cosT": cosT, "sinT": sinT, "bandT": bandT, "onesT": onesT,
        })
    res = run_bass_kernel_spmd(_PROG, in_maps, core_ids=list(range(NCORES)),
                               trace=trace, **kw)
    acc = np.zeros((ROWS, D), np.float64)
    for r in res.results:
        acc += np.asarray(r["out"], np.float32)
    return acc.astype(np.float32).reshape(B, S, D), res


def kernel(query, Wq, Wk, Wv, Wo):
    out, _ = _run(dict(query=query, Wq=Wq, Wk=Wk, Wv=Wv, Wo=Wo))
    return out



# revision 36
# speedup vs baseline: 1.0639x; 1.0608x over previous
"""Causal attention with RoPE on 8 Trainium2 NeuronCores.

Tensor-parallel over heads: core c owns heads [2c, 2c+2). Each core computes
its heads' Q/K/V projections (fp32r matmuls), RoPE, causal attention in a
transposed layout (keys on partitions), and a partial output projection
through its slice of Wo. The 8 partial outputs are summed on the host.

Layout notes:
  - x is passed transposed (xT [D, B*S]) so d_model lands on partitions for
    all projection matmuls.
  - Q/K are produced transposed (QT/KT [head_dim, S]); scores are computed
    transposed (scoresT [k, q]) so the attn@V contraction has keys on
    partitions for both operands. Softmax denominators come from an M=1
    matmul with a ones vector; normalization happens on the attention output
    tiles via a gpsimd partition-broadcast of 1/denom.
  - All matmul inputs are float32r (fp32 truncated to fp22 by the PE), which
    streams at full speed (1 cycle/row) instead of fp32's 1/4 rate.
  - The causal structure lets attention for query block qj start as soon as
    projections for column block cb=qj are done, so each iteration runs
    proj(cb) -> attention(qj=cb) -> output rows of qj; engines stay mixed
    and the DMA-paced warmup overlaps compute.
"""
import numpy as np

import concourse.bacc as bacc
import concourse.bass as bass
import concourse.bass_isa as bass_isa
import concourse.tile as tile
import concourse.mybir as mybir
from concourse.bass_utils import run_bass_kernel_spmd

AF = mybir.ActivationFunctionType
F32 = mybir.dt.float32
F32R = mybir.dt.float32r
BF16 = mybir.dt.bfloat16

P = 128            # partitions
DH = 128           # head dim
D = 2048           # d_model
S = 2048           # sequence length
B = 2              # batch
NCORES = 8
HL = 2             # heads per core
LF = HL * DH       # 256 local head features
KC = D // P        # 16 d_model chunks
NCB = S // 512     # 4 column blocks of 512 positions per batch
NKB = S // P       # 16 key blocks per batch
NNT = D // 512     # 4 output column tiles
ROWS = B * S
SCALE = float(1.0 / np.sqrt(DH))

_PROG = None
# timing-only probes (wrong numerics): 1 = halve x DMA, 2 = skip odd-kb exp,
# 3 = skip attention entirely
PROBE = 0


def _emit(nc, sbp, psp, t):
    xT, wqT, wkT, wvT, woT, cosT, sinT, bandT, out = (
        t["xT"], t["wqT"], t["wkT"], t["wvT"], t["woT"], t["cosT"], t["sinT"],
        t["bandT"], t["out"])

    # ---------------- constants ----------------
    wq = sbp.tile([P, KC * LF], BF16, name="wq")
    wk = sbp.tile([P, KC * LF], BF16, name="wk")
    wv = sbp.tile([P, KC * LF], BF16, name="wv")
    wo = sbp.tile([P, HL * D], BF16, name="wo")
    cos = sbp.tile([DH, S], F32, name="cos")
    sin = sbp.tile([DH, S], F32, name="sin")
    band = sbp.tile([P, 128], BF16, name="band")

    # DMA transfer bandwidth is shared, so order transfers by first use:
    # wq/wk group g feeds matmul chunk group g, x tiles interleave, and the
    # rest (cos/sin for rope, wv for the V pass, wo much later) follows.
    def emit_wqk(g):
        gk = slice(g * 4 * P, (g + 1) * 4 * P)
        nc.sync.dma_start(
            out=wq[:, g * 4 * LF:(g + 1) * 4 * LF],
            in_=wqT[gk, :].rearrange("(kc p) f -> p kc f", p=P))
        nc.scalar.dma_start(
            out=wk[:, g * 4 * LF:(g + 1) * 4 * LF],
            in_=wkT[gk, :].rearrange("(kc p) f -> p kc f", p=P))

    # chunk 0 of wq/wk alone first: the very first matmul needs only
    # wq[0]+wk[0]+x[0] (~0.5 MB), not the full first group
    nc.sync.dma_start(out=wq[:, 0:LF], in_=wqT[0:P, :])
    nc.scalar.dma_start(out=wk[:, 0:LF], in_=wkT[0:P, :])

    def emit_wqk_rest0():
        gk = slice(P, 4 * P)
        nc.sync.dma_start(
            out=wq[:, LF:4 * LF],
            in_=wqT[gk, :].rearrange("(kc p) f -> p kc f", p=P))
        nc.scalar.dma_start(
            out=wk[:, LF:4 * LF],
            in_=wkT[gk, :].rearrange("(kc p) f -> p kc f", p=P))

    nc.scalar.dma_start(out=cos, in_=cosT[:, :])
    nc.scalar.dma_start(out=sin, in_=sinT[:, :])

    def deferred_consts():
        nc.gpsimd.dma_start(out=band, in_=bandT[:, :])
        for g in range(4):
            gk = slice(g * 4 * P, (g + 1) * 4 * P)
            nc.gpsimd.dma_start(
                out=wv[:, g * 4 * LF:(g + 1) * 4 * LF],
                in_=wvT[gk, :].rearrange("(kc p) f -> p kc f", p=P))
        # wo in nt-halves so the first Wo matmuls (nt 0/1) unblock early
        for half in range(2):
            for h in range(HL):
                nc.gpsimd.dma_start(
                    out=wo[:, h * D + half * 1024: h * D + (half + 1) * 1024],
                    in_=woT[h * P:(h + 1) * P,
                            half * 1024:(half + 1) * 1024])

    emit_wo = _make_emit_wo(nc, sbp, psp, wo, out)
    emit_attn = _make_emit_attn(nc, sbp, psp, band)

    for b in range(B):
        qt = sbp.tile([P, HL * S], F32R, name=f"qt{b}", tag="qt")
        kt = sbp.tile([P, HL * S], F32R, name=f"kt{b}", tag="kt")
        vsb = sbp.tile([P, NKB * LF], BF16, name=f"v{b}", tag="v")
        ot = sbp.tile([P, HL * S], BF16, name=f"ot{b}", tag="ot")

        for cb in range(NCB):
            # ---------------- x loads ----------------
            xtg = []
            first = b == 0 and cb == 0
            ngrp = 2 if PROBE == 1 else 4
            for g in range(ngrp):
                xt = sbp.tile([P, 4 * 512], BF16, name=f"xt{b}_{cb}_{g}",
                              tag="xt", bufs=5)
                eng = nc.sync if g % 2 == 0 else nc.scalar
                src = xT[g * 4 * P:(g + 1) * 4 * P,
                         b * S + cb * 512: b * S + (cb + 1) * 512]
                if first and g == 0:
                    # split so the kc=0 slice lands first
                    nc.sync.dma_start(
                        out=xt[:, 0:512], in_=xT[0:P, 0:512])
                    emit_wqk_rest0()
                    nc.sync.dma_start(
                        out=xt[:, 512:2048],
                        in_=xT[P:4 * P, 0:512]
                            .rearrange("(kc p) s -> p kc s", p=P))
                else:
                    eng.dma_start(
                        out=xt,
                        in_=src.rearrange("(kc p) s -> p kc s", p=P))
                xtg.append(xt)
                if first and g < 3:
                    emit_wqk(g + 1)
            if first:
                deferred_consts()
            xts = [xtg[(kc // 4) % ngrp][:, (kc % 4) * 512:(kc % 4 + 1) * 512]
                   for kc in range(KC)]

            # ---------------- Q/K projections ----------------
            pqs = {}
            for key in ("q", "k"):
                for h in range(HL):
                    pqs[(key, h)] = psp.tile(
                        [P, 512], F32, name=f"p{key}{h}_{b}_{cb}",
                        tag=("po" if key == "q" else "pd"), bufs=2)
            for kc in range(KC):
                for key in ("q", "k"):
                    wsb = wq if key == "q" else wk
                    for h in range(HL):
                        nc.tensor.matmul(
                            pqs[(key, h)],
                            lhsT=wsb[:, kc * LF + h * DH: kc * LF + (h + 1) * DH],
                            rhs=xts[kc],
                            start=(kc == 0), stop=(kc == KC - 1))
            # RoPE drain: dst = pq*cos + rotate_half(pq)*sin (sin pre-signed)
            for key in ("q", "k"):
                dst = qt if key == "q" else kt
                for h in range(HL):
                    pq = pqs[(key, h)]
                    dsl = dst[:, h * S + cb * 512: h * S + (cb + 1) * 512]
                    cs = slice(cb * 512, (cb + 1) * 512)
                    ra = sbp.tile([P, 512], F32, name=f"ra{b}_{cb}_{key}{h}",
                                  tag="ra", bufs=4)
                    nc.vector.tensor_mul(ra, pq, cos[:, cs])
                    nc.vector.tensor_mul(dsl[0:64, :], pq[64:128, :],
                                         sin[0:64, cs])
                    nc.vector.tensor_mul(dsl[64:128, :], pq[0:64, :],
                                         sin[64:128, cs])
                    nc.vector.tensor_add(dsl, dsl.bitcast(F32), ra)

            # ---------------- V projection (natural layout) ----------------
            pvs = [psp.tile([P, LF], F32, name=f"pv{b}_{cb}_{r}", tag="ps",
                            bufs=4)
                   for r in range(4)]
            for kc in range(KC):
                for r in range(4):
                    nc.tensor.matmul(
                        pvs[r],
                        lhsT=xts[kc][:, r * P:(r + 1) * P],
                        rhs=wv[:, kc * LF:(kc + 1) * LF],
                        start=(kc == 0), stop=(kc == KC - 1))
            for r in range(4):
                kb = cb * 4 + r
                nc.scalar.copy(vsb[:, kb * LF:(kb + 1) * LF], pvs[r])
            if PROBE == 3:
                continue

            # attention + partial output projection for this query block
            if PROBE != 3:
                emit_attn(b, cb, qt, kt, vsb, ot)
                emit_wo(b, cb, ot)


def _make_emit_attn(nc, sbp, psp, band):
    def emit_attn(b, qj, qt, kt, vsb, ot):
        # Both heads interleaved: each head's exp latency hides behind the
        # other head's matmuls.
        nkb = 4 * qj + 4
        po = {}
        acc = {}
        for h in range(HL):
            po[h] = psp.tile([P, 512], F32, name=f"po{b}_{h}_{qj}",
                             tag="po", bufs=2)
            # running key-sum of exp (DVE); partition-reduced once at the end
            acc[h] = sbp.tile([P, 512], F32, name=f"ac{b}_{h}_{qj}",
                              tag="ac", bufs=2)
        exs = {}

        # For diagonal block r (kb = 4*qj + r), query columns [0, r*128) see
        # only masked keys in this block: skip them entirely — the scores
        # matmul, exp, attn@V and denominator all run on cols [r*128, 512).
        # The kb==0 matmuls always cover the full range (off=0 there), so
        # the accumulation start clears the whole bank.
        def _off(kb):
            return max(0, kb - 4 * qj) * P

        def emit_sc(h, kb):
            off = _off(kb)
            pss = psp.tile([P, 512], F32, name=f"pss{b}_{h}_{qj}_{kb}",
                           tag="ps", bufs=4)
            nc.tensor.matmul(
                pss[:, off:512],
                lhsT=kt[:, h * S + kb * P: h * S + (kb + 1) * P],
                rhs=qt[:, h * S + qj * 512 + off: h * S + (qj + 1) * 512],
                start=True, stop=True)
            ex = sbp.tile([P, 512], BF16, name=f"ex{b}_{h}_{qj}_{kb}",
                          tag="ex", bufs=6)
            nc.scalar.activation(ex[:, off:512], pss[:, off:512], AF.Exp,
                                 scale=SCALE)
            if kb >= 4 * qj:
                # upper-triangle mask on the diagonal 128-block
                nc.vector.tensor_mul(
                    ex[:, off:off + P], ex[:, off:off + P],
                    band[:, 0:128])
            exs[(h, kb)] = ex

        def emit_av(h, kb, last):
            off = _off(kb)
            nc.tensor.matmul(
                po[h][:, off:512],
                lhsT=vsb[:, kb * LF + h * DH: kb * LF + h * DH + DH],
                rhs=exs[(h, kb)][:, off:512], start=(kb == 0), stop=last)
            # denominator accumulate on DVE (columns < off see only masked
            # keys in this block, so skipping them keeps acc correct)
            if kb == 0:
                nc.vector.tensor_copy(acc[h], exs[(h, kb)])
            else:
                nc.vector.tensor_add(
                    acc[h][:, off:512], acc[h][:, off:512],
                    exs[(h, kb)][:, off:512])

        emit_sc(0, 0)
        emit_sc(1, 0)
        for kb in range(nkb):
            for h in range(HL):
                if kb + 1 < nkb:
                    emit_sc(h, kb + 1)
                emit_av(h, kb, last=(kb == nkb - 1))

        for h in range(HL):
            ar = sbp.tile([P, 512], F32, name=f"ar{b}_{h}_{qj}",
                          tag="ar", bufs=2)
            nc.gpsimd.partition_all_reduce(ar, acc[h], channels=P,
                                           reduce_op=bass_isa.ReduceOp.add)
            nc.vector.reciprocal(ar, ar)
            nc.vector.tensor_mul(
                ot[:, h * S + qj * 512: h * S + (qj + 1) * 512], po[h], ar)
    return emit_attn


def _make_emit_wo(nc, sbp, psp, wo, out):
    def emit_wo(b, qj, ot):
        for qc in range(4 * qj, 4 * qj + 4):
            st = None
            for nt in range(NNT):
                pw = psp.tile([P, 512], F32, name=f"pw{b}_{qc}_{nt}",
                              tag="ps", bufs=4)
                for h in range(HL):
                    nc.tensor.matmul(
                        pw,
                        lhsT=ot[:, h * S + qc * P: h * S + (qc + 1) * P],
                        rhs=wo[:, h * D + nt * 512: h * D + (nt + 1) * 512],
                        start=(h == 0), stop=(h == HL - 1))
                if nt % 2 == 0:
                    st = sbp.tile([P, 1024], BF16, name=f"st{b}_{qc}_{nt}",
                                  tag="st", bufs=2)
                    nc.scalar.copy(st[:, 0:512], pw)
                else:
                    nc.vector.tensor_copy(st[:, 512:1024], pw)
                    nc.sync.dma_start(
                        out=out[b * S + qc * P: b * S + (qc + 1) * P,
                                (nt - 1) * 512:(nt + 1) * 512],
                        in_=st)
    return emit_wo


def _build(loop_n=0):
    nc = bacc.Bacc("TRN2", target_bir_lowering=False, debug=False)
    t = {}
    t["xT"] = nc.dram_tensor("xT", [D, ROWS], BF16, kind="ExternalInput")
    t["wqT"] = nc.dram_tensor("wqT", [D, LF], BF16, kind="ExternalInput")
    t["wkT"] = nc.dram_tensor("wkT", [D, LF], BF16, kind="ExternalInput")
    t["wvT"] = nc.dram_tensor("wvT", [D, LF], BF16, kind="ExternalInput")
    t["woT"] = nc.dram_tensor("woT", [LF, D], BF16, kind="ExternalInput")
    t["cosT"] = nc.dram_tensor("cosT", [DH, S], F32, kind="ExternalInput")
    t["sinT"] = nc.dram_tensor("sinT", [DH, S], F32, kind="ExternalInput")
    t["bandT"] = nc.dram_tensor("bandT", [P, 128], BF16, kind="ExternalInput")
    t["out"] = nc.dram_tensor("out", [ROWS, D], BF16, kind="ExternalOutput")
    with tile.TileContext(nc) as tc:
        with tc.tile_pool(name="sb", bufs=1) as sbp, \
             tc.tile_pool(name="ps", bufs=4, space="PSUM") as psp:
            if loop_n:
                with tc.For_i(0, loop_n, 1,
                              hint_engines=(mybir.EngineType.PE,
                                            mybir.EngineType.Activation,
                                            mybir.EngineType.DVE)):
                    _emit(nc, sbp, psp, t)
            else:
                _emit(nc, sbp, psp, t)
    nc.compile()
    return nc


def _tables():
    half = np.arange(0, DH, 2, dtype=np.float32) / np.float32(DH)
    inv_freq = (np.float32(1.0) / (np.float32(10000.0) ** half)).astype(np.float32)
    pos = np.arange(S, dtype=np.float32)
    freqs = np.outer(pos, inv_freq).astype(np.float32)        # [S, 64]
    emb = np.concatenate([freqs, freqs], axis=1)              # [S, DH]
    cosT = np.ascontiguousarray(np.cos(emb).T).astype(np.float32)
    sinT = np.sin(emb).T.astype(np.float32).copy()
    sinT[0:DH // 2, :] *= np.float32(-1.0)                    # pre-signed
    sinT = np.ascontiguousarray(sinT)
    # band[kl, c] = 1 iff c >= kl: the upper-triangle mask every diagonal
    # 128-block uses (query col off+c vs key row off+kl differ by c-kl)
    kl = np.arange(P)[:, None]
    c = np.arange(P)[None, :]
    import ml_dtypes
    bandT = (c >= kl).astype(ml_dtypes.bfloat16)
    onesT = np.ones((P, 1), np.float32)
    return cosT, sinT, bandT, onesT


def _in_maps(inputs):
    import ml_dtypes
    bf = ml_dtypes.bfloat16
    q = np.asarray(inputs["query"], dtype=np.float32)
    Wq = np.asarray(inputs["Wq"], dtype=np.float32)
    Wk = np.asarray(inputs["Wk"], dtype=np.float32)
    Wv = np.asarray(inputs["Wv"], dtype=np.float32)
    Wo = np.asarray(inputs["Wo"], dtype=np.float32)
    xT = np.ascontiguousarray(q.reshape(ROWS, D).T.astype(bf))
    cosT, sinT, bandT, onesT = _tables()
    in_maps = []
    for ci in range(NCORES):
        rs = slice(ci * LF, (ci + 1) * LF)
        in_maps.append({
            "xT": xT,
            "wqT": np.ascontiguousarray(Wq[rs, :].T.astype(bf)),
            "wkT": np.ascontiguousarray(Wk[rs, :].T.astype(bf)),
            "wvT": np.ascontiguousarray(Wv[rs, :].T.astype(bf)),
            "woT": np.ascontiguousarray(Wo[:, rs].T.astype(bf)),
            "cosT": cosT, "sinT": sinT, "bandT": bandT,
        })
    return in_maps


def _run(inputs, trace=False, **kw):
    global _PROG
    if _PROG is None:
        _PROG = _build()
    res = run_bass_kernel_spmd(_PROG, _in_maps(inputs),
                               core_ids=list(range(NCORES)),
                               trace=trace, **kw)
    acc = np.zeros((ROWS, D), np.float64)
    for r in res.results:
        acc += np.asarray(r["out"], np.float32)
    return acc.astype(np.float32).reshape(B, S, D), res


def kernel(query, Wq, Wk, Wv, Wo):
    out, _ = _run(dict(query=query, Wq=Wq, Wk=Wk, Wv=Wv, Wo=Wo))
    return out



# revision 40
# speedup vs baseline: 1.0708x; 1.0065x over previous
"""Causal attention with RoPE on 8 Trainium2 NeuronCores.

Tensor-parallel over heads: core c owns heads [2c, 2c+2). Each core computes
its heads' Q/K/V projections (fp32r matmuls), RoPE, causal attention in a
transposed layout (keys on partitions), and a partial output projection
through its slice of Wo. The 8 partial outputs are summed on the host.

Layout notes:
  - x is passed transposed (xT [D, B*S]) so d_model lands on partitions for
    all projection matmuls.
  - Q/K are produced transposed (QT/KT [head_dim, S]); scores are computed
    transposed (scoresT [k, q]) so the attn@V contraction has keys on
    partitions for both operands. Softmax denominators come from an M=1
    matmul with a ones vector; normalization happens on the attention output
    tiles via a gpsimd partition-broadcast of 1/denom.
  - All matmul inputs are float32r (fp32 truncated to fp22 by the PE), which
    streams at full speed (1 cycle/row) instead of fp32's 1/4 rate.
  - The causal structure lets attention for query block qj start as soon as
    projections for column block cb=qj are done, so each iteration runs
    proj(cb) -> attention(qj=cb) -> output rows of qj; engines stay mixed
    and the DMA-paced warmup overlaps compute.
"""
import numpy as np

import concourse.bacc as bacc
import concourse.bass as bass
import concourse.bass_isa as bass_isa
import concourse.tile as tile
import concourse.mybir as mybir
from concourse.bass_utils import run_bass_kernel_spmd

AF = mybir.ActivationFunctionType
F32 = mybir.dt.float32
F32R = mybir.dt.float32r
BF16 = mybir.dt.bfloat16

P = 128            # partitions
DH = 128           # head dim
D = 2048           # d_model
S = 2048           # sequence length
B = 2              # batch
NCORES = 8
HL = 2             # heads per core
LF = HL * DH       # 256 local head features
KC = D // P        # 16 d_model chunks
NCB = S // 512     # 4 column blocks of 512 positions per batch
NKB = S // P       # 16 key blocks per batch
NNT = D // 512     # 4 output column tiles
ROWS = B * S
SCALE = float(1.0 / np.sqrt(DH))

_PROG = None
# timing-only probes (wrong numerics): 1 = halve x DMA, 2 = skip odd-kb exp,
# 3 = skip attention entirely
PROBE = 0


def _emit(nc, sbp, psp, t):
    xT, wqT, wkT, wvT, woT, cosT, sinT, bandT, out = (
        t["xT"], t["wqT"], t["wkT"], t["wvT"], t["woT"], t["cosT"], t["sinT"],
        t["bandT"], t["out"])

    # ---------------- constants ----------------
    wq = sbp.tile([P, KC * LF], BF16, name="wq")
    wk = sbp.tile([P, KC * LF], BF16, name="wk")
    wv = sbp.tile([P, KC * LF], BF16, name="wv")
    wo = sbp.tile([P, HL * D], BF16, name="wo")
    cos = sbp.tile([DH, S], F32, name="cos")
    sin = sbp.tile([DH, S], F32, name="sin")
    band = sbp.tile([P, 128], BF16, name="band")

    # DMA transfer bandwidth is shared, so order transfers by first use:
    # wq/wk group g feeds matmul chunk group g, x tiles interleave, and the
    # rest (cos/sin for rope, wv for the V pass, wo much later) follows.
    def emit_wqk(g):
        gk = slice(g * 4 * P, (g + 1) * 4 * P)
        nc.sync.dma_start(
            out=wq[:, g * 4 * LF:(g + 1) * 4 * LF],
            in_=wqT[gk, :].rearrange("(kc p) f -> p kc f", p=P))
        nc.scalar.dma_start(
            out=wk[:, g * 4 * LF:(g + 1) * 4 * LF],
            in_=wkT[gk, :].rearrange("(kc p) f -> p kc f", p=P))

    # chunk 0 of wq/wk alone first: the very first matmul needs only
    # wq[0]+wk[0]+x[0] (~0.5 MB), not the full first group
    nc.sync.dma_start(out=wq[:, 0:LF], in_=wqT[0:P, :])
    nc.scalar.dma_start(out=wk[:, 0:LF], in_=wkT[0:P, :])

    def emit_wqk_rest0():
        gk = slice(P, 4 * P)
        nc.sync.dma_start(
            out=wq[:, LF:4 * LF],
            in_=wqT[gk, :].rearrange("(kc p) f -> p kc f", p=P))
        nc.scalar.dma_start(
            out=wk[:, LF:4 * LF],
            in_=wkT[gk, :].rearrange("(kc p) f -> p kc f", p=P))

    nc.scalar.dma_start(out=cos, in_=cosT[:, :])
    nc.scalar.dma_start(out=sin, in_=sinT[:, :])

    def deferred_consts():
        nc.gpsimd.dma_start(out=band, in_=bandT[:, :])
        for g in range(4):
            gk = slice(g * 4 * P, (g + 1) * 4 * P)
            nc.gpsimd.dma_start(
                out=wv[:, g * 4 * LF:(g + 1) * 4 * LF],
                in_=wvT[gk, :].rearrange("(kc p) f -> p kc f", p=P))
        # wo in nt-halves so the first Wo matmuls (nt 0/1) unblock early
        for half in range(2):
            for h in range(HL):
                nc.gpsimd.dma_start(
                    out=wo[:, h * D + half * 1024: h * D + (half + 1) * 1024],
                    in_=woT[h * P:(h + 1) * P,
                            half * 1024:(half + 1) * 1024])

    emit_wo = _make_emit_wo(nc, sbp, psp, wo, out)
    emit_attn = _make_emit_attn(nc, sbp, psp, band)

    for b in range(B):
        qt = sbp.tile([P, HL * S], F32R, name=f"qt{b}", tag="qt")
        kt = sbp.tile([P, HL * S], F32R, name=f"kt{b}", tag="kt")
        vsb = sbp.tile([P, NKB * LF], BF16, name=f"v{b}", tag="v")
        ot = sbp.tile([P, HL * S], BF16, name=f"ot{b}", tag="ot")

        for cb in range(NCB):
            # ---------------- x loads ----------------
            xtg = []
            first = b == 0 and cb == 0
            ngrp = 2 if PROBE == 1 else 4
            for g in range(ngrp):
                xt = sbp.tile([P, 4 * 512], BF16, name=f"xt{b}_{cb}_{g}",
                              tag="xt", bufs=8)
                eng = nc.sync if g % 2 == 0 else nc.scalar
                src = xT[g * 4 * P:(g + 1) * 4 * P,
                         b * S + cb * 512: b * S + (cb + 1) * 512]
                if first and g == 0:
                    # split so the kc=0 slice lands first
                    nc.sync.dma_start(
                        out=xt[:, 0:512], in_=xT[0:P, 0:512])
                    emit_wqk_rest0()
                    nc.sync.dma_start(
                        out=xt[:, 512:2048],
                        in_=xT[P:4 * P, 0:512]
                            .rearrange("(kc p) s -> p kc s", p=P))
                else:
                    eng.dma_start(
                        out=xt,
                        in_=src.rearrange("(kc p) s -> p kc s", p=P))
                xtg.append(xt)
                if first and g < 3:
                    emit_wqk(g + 1)
            if first:
                deferred_consts()
            xts = [xtg[(kc // 4) % ngrp][:, (kc % 4) * 512:(kc % 4 + 1) * 512]
                   for kc in range(KC)]

            # ---------------- Q/K projections ----------------
            pqs = {}
            for key in ("q", "k"):
                for h in range(HL):
                    pqs[(key, h)] = psp.tile(
                        [P, 512], F32, name=f"p{key}{h}_{b}_{cb}",
                        tag=("po" if key == "q" else "pd"), bufs=2)
            for kc in range(KC):
                for key in ("q", "k"):
                    wsb = wq if key == "q" else wk
                    for h in range(HL):
                        nc.tensor.matmul(
                            pqs[(key, h)],
                            lhsT=wsb[:, kc * LF + h * DH: kc * LF + (h + 1) * DH],
                            rhs=xts[kc],
                            start=(kc == 0), stop=(kc == KC - 1))
            # RoPE drain: dst = pq*cos + rotate_half(pq)*sin (sin pre-signed).
            # Act rotates halves PSUM->SBUF (frees the PSUM slot fast — the
            # next cb's matmuls stall on it); DVE then does 3 ops per tile.
            for key in ("q", "k"):
                dst = qt if key == "q" else kt
                for h in range(HL):
                    pq = pqs[(key, h)]
                    pqr = sbp.tile([P, 512], F32, name=f"pq{b}_{cb}_{key}{h}",
                                   tag="pqs", bufs=4)
                    nc.scalar.copy(pqr[0:64, :], pq[64:128, :])
                    nc.scalar.copy(pqr[64:128, :], pq[0:64, :])
                    dsl = dst[:, h * S + cb * 512: h * S + (cb + 1) * 512]
                    cs = slice(cb * 512, (cb + 1) * 512)
                    ra = sbp.tile([P, 512], F32, name=f"ra{b}_{cb}_{key}{h}",
                                  tag="ra", bufs=4)
                    nc.vector.tensor_mul(ra, pq, cos[:, cs])
                    nc.vector.tensor_mul(dsl, pqr, sin[:, cs])
                    nc.vector.tensor_add(dsl, dsl.bitcast(F32), ra)

            # ---------------- V projection (natural layout) ----------------
            pvs = [psp.tile([P, LF], F32, name=f"pv{b}_{cb}_{r}", tag="ps",
                            bufs=4)
                   for r in range(4)]
            for kc in range(KC):
                for r in range(4):
                    nc.tensor.matmul(
                        pvs[r],
                        lhsT=xts[kc][:, r * P:(r + 1) * P],
                        rhs=wv[:, kc * LF:(kc + 1) * LF],
                        start=(kc == 0), stop=(kc == KC - 1))
            for r in range(4):
                kb = cb * 4 + r
                nc.scalar.copy(vsb[:, kb * LF:(kb + 1) * LF], pvs[r])
            if PROBE == 3:
                continue

            # attention + partial output projection for this query block
            if PROBE != 3:
                emit_attn(b, cb, qt, kt, vsb, ot)
                if PROBE != 4:
                    emit_wo(b, cb, ot)


def _make_emit_attn(nc, sbp, psp, band):
    def emit_attn(b, qj, qt, kt, vsb, ot):
        # Both heads interleaved: each head's exp latency hides behind the
        # other head's matmuls.
        nkb = 4 * qj + 4
        po = {}
        acc = {}
        for h in range(HL):
            po[h] = psp.tile([P, 512], F32, name=f"po{b}_{h}_{qj}",
                             tag="po", bufs=2)
            # running key-sum of exp (DVE); partition-reduced once at the end
            acc[h] = sbp.tile([P, 512], F32, name=f"ac{b}_{h}_{qj}",
                              tag="ac", bufs=2)
        exs = {}

        # For diagonal block r (kb = 4*qj + r), query columns [0, r*128) see
        # only masked keys in this block: skip them entirely — the scores
        # matmul, exp, attn@V and denominator all run on cols [r*128, 512).
        # The kb==0 matmuls always cover the full range (off=0 there), so
        # the accumulation start clears the whole bank.
        def _off(kb):
            return max(0, kb - 4 * qj) * P

        def emit_sc(h, kb):
            off = _off(kb)
            pss = psp.tile([P, 512], F32, name=f"pss{b}_{h}_{qj}_{kb}",
                           tag="ps", bufs=4)
            nc.tensor.matmul(
                pss[:, off:512],
                lhsT=kt[:, h * S + kb * P: h * S + (kb + 1) * P],
                rhs=qt[:, h * S + qj * 512 + off: h * S + (qj + 1) * 512],
                start=True, stop=True)
            ex = sbp.tile([P, 512], BF16, name=f"ex{b}_{h}_{qj}_{kb}",
                          tag="ex", bufs=6)
            nc.scalar.activation(ex[:, off:512], pss[:, off:512], AF.Exp,
                                 scale=SCALE)
            if kb >= 4 * qj:
                # upper-triangle mask on the diagonal 128-block
                nc.vector.tensor_mul(
                    ex[:, off:off + P], ex[:, off:off + P],
                    band[:, 0:128])
            exs[(h, kb)] = ex

        def emit_av(h, kb, last):
            off = _off(kb)
            nc.tensor.matmul(
                po[h][:, off:512],
                lhsT=vsb[:, kb * LF + h * DH: kb * LF + h * DH + DH],
                rhs=exs[(h, kb)][:, off:512], start=(kb == 0), stop=last)
            # denominator accumulate on DVE (columns < off see only masked
            # keys in this block, so skipping them keeps acc correct)
            if kb == 0:
                nc.vector.tensor_copy(acc[h], exs[(h, kb)])
            else:
                nc.vector.tensor_add(
                    acc[h][:, off:512], acc[h][:, off:512],
                    exs[(h, kb)][:, off:512])

        emit_sc(0, 0)
        emit_sc(1, 0)
        for kb in range(nkb):
            for h in range(HL):
                if kb + 1 < nkb:
                    emit_sc(h, kb + 1)
                emit_av(h, kb, last=(kb == nkb - 1))

        for h in range(HL):
            ar = sbp.tile([P, 512], F32, name=f"ar{b}_{h}_{qj}",
                          tag="ar", bufs=2)
            nc.gpsimd.partition_all_reduce(ar, acc[h], channels=P,
                                           reduce_op=bass_isa.ReduceOp.add)
            nc.vector.reciprocal(ar, ar)
            nc.vector.tensor_mul(
                ot[:, h * S + qj * 512: h * S + (qj + 1) * 512], po[h], ar)
    return emit_attn


def _make_emit_wo(nc, sbp, psp, wo, out):
    def emit_wo(b, qj, ot):
        for qc in range(4 * qj, 4 * qj + 4):
            st = None
            for nt in range(NNT):
                pw = psp.tile([P, 512], F32, name=f"pw{b}_{qc}_{nt}",
                              tag="ps", bufs=4)
                for h in range(HL):
                    nc.tensor.matmul(
                        pw,
                        lhsT=ot[:, h * S + qc * P: h * S + (qc + 1) * P],
                        rhs=wo[:, h * D + nt * 512: h * D + (nt + 1) * 512],
                        start=(h == 0), stop=(h == HL - 1))
                if nt % 2 == 0:
                    st = sbp.tile([P, 1024], BF16, name=f"st{b}_{qc}_{nt}",
                                  tag="st", bufs=2)
                    nc.scalar.copy(st[:, 0:512], pw)
                else:
                    nc.vector.tensor_copy(st[:, 512:1024], pw)
                    nc.sync.dma_start(
                        out=out[b * S + qc * P: b * S + (qc + 1) * P,
                                (nt - 1) * 512:(nt + 1) * 512],
                        in_=st)
    return emit_wo


def _build(loop_n=0):
    nc = bacc.Bacc("TRN2", target_bir_lowering=False, debug=False)
    t = {}
    t["xT"] = nc.dram_tensor("xT", [D, ROWS], BF16, kind="ExternalInput")
    t["wqT"] = nc.dram_tensor("wqT", [D, LF], BF16, kind="ExternalInput")
    t["wkT"] = nc.dram_tensor("wkT", [D, LF], BF16, kind="ExternalInput")
    t["wvT"] = nc.dram_tensor("wvT", [D, LF], BF16, kind="ExternalInput")
    t["woT"] = nc.dram_tensor("woT", [LF, D], BF16, kind="ExternalInput")
    t["cosT"] = nc.dram_tensor("cosT", [DH, S], F32, kind="ExternalInput")
    t["sinT"] = nc.dram_tensor("sinT", [DH, S], F32, kind="ExternalInput")
    t["bandT"] = nc.dram_tensor("bandT", [P, 128], BF16, kind="ExternalInput")
    t["out"] = nc.dram_tensor("out", [ROWS, D], BF16, kind="ExternalOutput")
    with tile.TileContext(nc) as tc:
        with tc.tile_pool(name="sb", bufs=1) as sbp, \
             tc.tile_pool(name="ps", bufs=4, space="PSUM") as psp:
            if loop_n:
                with tc.For_i(0, loop_n, 1,
                              hint_engines=(mybir.EngineType.PE,
                                            mybir.EngineType.Activation,
                                            mybir.EngineType.DVE)):
                    _emit(nc, sbp, psp, t)
            else:
                _emit(nc, sbp, psp, t)
    nc.compile()
    return nc


def _tables():
    half = np.arange(0, DH, 2, dtype=np.float32) / np.float32(DH)
    inv_freq = (np.float32(1.0) / (np.float32(10000.0) ** half)).astype(np.float32)
    pos = np.arange(S, dtype=np.float32)
    freqs = np.outer(pos, inv_freq).astype(np.float32)        # [S, 64]
    emb = np.concatenate([freqs, freqs], axis=1)              # [S, DH]
    cosT = np.ascontiguousarray(np.cos(emb).T).astype(np.float32)
    sinT = np.sin(emb).T.astype(np.float32).copy()
    sinT[0:DH // 2, :] *= np.float32(-1.0)                    # pre-signed
    sinT = np.ascontiguousarray(sinT)
    # band[kl, c] = 1 iff c >= kl: the upper-triangle mask every diagonal
    # 128-block uses (query col off+c vs key row off+kl differ by c-kl)
    kl = np.arange(P)[:, None]
    c = np.arange(P)[None, :]
    import ml_dtypes
    bandT = (c >= kl).astype(ml_dtypes.bfloat16)
    onesT = np.ones((P, 1), np.float32)
    return cosT, sinT, bandT, onesT


def _in_maps(inputs):
    import ml_dtypes
    bf = ml_dtypes.bfloat16
    q = np.asarray(inputs["query"], dtype=np.float32)
    Wq = np.asarray(inputs["Wq"], dtype=np.float32)
    Wk = np.asarray(inputs["Wk"], dtype=np.float32)
    Wv = np.asarray(inputs["Wv"], dtype=np.float32)
    Wo = np.asarray(inputs["Wo"], dtype=np.float32)
    xT = np.ascontiguousarray(q.reshape(ROWS, D).T.astype(bf))
    cosT, sinT, bandT, onesT = _tables()
    in_maps = []
    for ci in range(NCORES):
        rs = slice(ci * LF, (ci + 1) * LF)
        in_maps.append({
            "xT": xT,
            "wqT": np.ascontiguousarray(Wq[rs, :].T.astype(bf)),
            "wkT": np.ascontiguousarray(Wk[rs, :].T.astype(bf)),
            "wvT": np.ascontiguousarray(Wv[rs, :].T.astype(bf)),
            "woT": np.ascontiguousarray(Wo[:, rs].T.astype(bf)),
            "cosT": cosT, "sinT": sinT, "bandT": bandT,
        })
    return in_maps


def _run(inputs, trace=False, **kw):
    global _PROG
    if _PROG is None:
        _PROG = _build()
    res = run_bass_kernel_spmd(_PROG, _in_maps(inputs),
                               core_ids=list(range(NCORES)),
                               trace=trace, **kw)
    acc = np.zeros((ROWS, D), np.float64)
    for r in res.results:
        acc += np.asarray(r["out"], np.float32)
    return acc.astype(np.float32).reshape(B, S, D), res


def kernel(query, Wq, Wk, Wv, Wo):
    out, _ = _run(dict(query=query, Wq=Wq, Wk=Wk, Wv=Wv, Wo=Wo))
    return out

